# revision 27
# baseline (speedup 1.0000x reference)
"""Trainium2 Bass kernel for nn_CrossAttention (linear attention, elu+1 feature map).

Math (per batch element n of B=4, sequence L = V*HW = 20480, C=256, H=8 heads, d=32):
    qkv = xb @ W_qkv ; q,k,v splits
    phi(t) = elu(t)+1  (exactly min(max(t+1,1), exp(t)))
    kv[h,m,d] = sum_l phi(k)[l,h,d] * v[l,h,m]
    z[l,h]   = 1 / (phi(q)[l,h,:] . sum_l phi(k)[l,h,:] + eps)
    y[l,h,m] = phi(q)[l,h,:] . kv[h,:,m] * z[l,h]
    out      = y @ W_proj + b_proj

Sharding: 8 cores = 4 batches x 2 L-halves (LH=10240 rows each). The only
cross-core data is the tiny kv/ksum partial ([2,128,130], bf16 = 66.5KB),
AllReduced over core pairs mid-kernel, hidden under phase-2 q-projection.

Perf model (per-engine ns per unit, PE at 2.4GHz full p-state):
  P1/i(256 l): T 1070 | Act ~1030 | DVE ~1060         -> tensor-bound
  C/chunk(512 l): T 853 | Act ~1000 | DVE ~1520       -> DVE-bound (hides AR)
  D/chunk: T 1707 | Act ~1700 | DVE ~1780             -> ~tensor-bound
(Pool/GPSIMD can't run min/max tensor ops through this walrus and can't
touch PSUM, so it only does memsets/DMA triggers/the collective.)
The previous version was 3-way balanced (tensor/vector/scalar all ~65% busy)
which kept the PE p-state at ~1.2GHz; this version keeps PE the strict
bottleneck in P1/D so it ramps to and holds 2.4GHz.

Key structural change vs v1: kv is folded into W_proj once per kernel
(M[feat,c'] = sum_m kv[h(feat),m,d(feat)] * Wp[(h,m),c'], built with one PE
transpose + one matmul after the AllReduce). Phase D then needs only:
  den (thin [8,512] via ksT lhsT) -> recip on 8 rows (Act, table op)
  -> z broadcast to 128 rows via mask4 matmul (PE, keeps it busy)
  -> ys = phiq * zb (DVE) -> out = ysT.T @ M (PE) -> psum->sbuf (Act) -> DMA.
This removes the y-matmul and halves Act work in D.

All matmuls bf16 (fp8 fails: quantization noise on the value path doesn't
average out through the cancelling sums). Reciprocal via the 1-instruction
approx Act table op (~1.2e-5 rel err measured on HW; tol here is 2e-2).
"""

import os
import sys
import numpy as np

if "/opt/trn_rl_repo" not in sys.path:
    sys.path.insert(0, "/opt/trn_rl_repo")

# ---------------- problem constants (hardcoded per contest rules) -----------
BV, HW, C = 20, 4096, 256
NVIEW = 5
B = BV // NVIEW          # 4
H = 8
D = C // H               # 32
L = NVIEW * HW           # 20480
N_CORES = 8
LH = L // 2              # 10240 rows per core
EPS = 1e-6               # folded away: den >> 1e-6 always (phi>0, ksum~2e4)

_NC_CACHE = {}


def _build_nc(lh=LH, with_bias=False, collective=True, split_waits=True,
              repeat=1):
    """Build the Bass program (SPMD, one core's share: [C, lh] -> [lh, C]).

    repeat>1 re-runs the whole body (incl. x DMA-in / out DMA) that many
    times in one launch; used by test.py to amortize launch overhead when
    timing. kernel() uses repeat=1.
    """
    import concourse.bass as bass
    import concourse.mybir as mybir
    import concourse.tile as tile
    from contextlib import ExitStack

    f32 = mybir.dt.float32
    bf16 = mybir.dt.bfloat16
    AF = mybir.ActivationFunctionType
    OP = mybir.AluOpType
    PSUM = bass.MemorySpace.PSUM
    DRAM = bass.MemorySpace.DRAM

    assert lh % 512 == 0
    NT2 = lh // 512          # phase-2 chunks == stash tiles per c-half (20)
    NT1 = lh // 256          # phase-1 iterations (2 l-tiles of 128 each) (40)
    BUILDC = max(NT2 - 3, 0)  # emit M-build after this C chunk (AR is done)

    nc = bass.Bass("TRN2", target_bir_lowering=False, debug=False,
                   num_devices=N_CORES)

    xT = nc.dram_tensor("xT", [C, lh], bf16, kind="ExternalInput")
    wqkv = nc.dram_tensor("w_qkv", [C, 3 * C], bf16, kind="ExternalInput")
    wproj = nc.dram_tensor("w_proj", [C, C], bf16, kind="ExternalInput")
    bproj = nc.dram_tensor("b_proj", [1, C], bf16, kind="ExternalInput")
    eye32 = nc.dram_tensor("eye32", [128, 128], bf16, kind="ExternalInput")
    eye128 = nc.dram_tensor("eye128", [128, 128], bf16, kind="ExternalInput")
    # bf16 output halves the ~10.5MB/core output DMA; costs ~0.4% element
    # rounding against a 2e-2 tolerance
    out = nc.dram_tensor("out", [lh, C], bf16, kind="ExternalOutput")

    # out rows l = c*512 + j*128 + p  ->  [NT2, 128, 4, 256]
    out_r = out[:].rearrange("(c j p) f -> c p j f", j=4, p=128)

    with tile.TileContext(nc) as tc, ExitStack() as ctx:
        const = ctx.enter_context(tc.tile_pool(name="const", bufs=1))
        stash = ctx.enter_context(tc.tile_pool(name="stash", bufs=1))
        dram = ctx.enter_context(tc.tile_pool(name="dram", bufs=1, space=DRAM))

        # ---- constants (loaded once, reused across repeats) ----------------
        # all constants on the gpsimd queue: the sync/scalar queues must
        # start pumping x immediately (P1 starves otherwise)
        w_sb = [const.tile([128, 3 * C], bf16, tag=f"w{h}", name=f"w{h}")
                for h in range(2)]
        for h in range(2):
            nc.gpsimd.dma_start(w_sb[h][:], wqkv[128 * h:128 * (h + 1), :])
        wp_sb = [const.tile([128, C], bf16, tag=f"wp{m}", name=f"wp{m}")
                 for m in range(2)]
        for m in range(2):
            nc.gpsimd.dma_start(wp_sb[m][:], wproj[128 * m:128 * (m + 1), :])
        eye_sb = const.tile([128, 128], bf16, tag="eye")
        nc.gpsimd.dma_start(eye_sb[:], eye32[:, :])
        eyeid_sb = const.tile([128, 128], bf16, tag="eyeid")
        nc.gpsimd.dma_start(eyeid_sb[:], eye128[:, :])
        if with_bias:
            brow = const.tile([1, C], bf16, tag="brow")
            nc.gpsimd.dma_start(brow[:], bproj[:, :])
            ones_k1 = const.tile([1, 128], bf16, tag="ones_k1")
            nc.gpsimd.memset(ones_k1[:], 1.0)

        # x^T stash tiles (re-DMA'd each repeat)
        xst = [[stash.tile([128, 512], bf16, tag=f"x{h}_{t}", name=f"x{h}_{t}")
                for t in range(NT2)] for h in range(2)]

        # v rhs staging: [slot, j, m-block, 130]; cols 128:130 of each block
        # stay 1.0 (ksum columns), data cols rewritten each iteration.
        vbuf = stash.tile([128, 4, 2, 2, 130], bf16, tag="vbuf", name="vbuf")
        nc.gpsimd.memset(vbuf[:], 1.0)

        for rep in range(repeat):
            # t-major order: the two c-halves of tile t arrive back-to-back,
            # so iteration 0 (which needs xst[0][0] AND xst[1][0]) isn't
            # stuck behind 20 unrelated transfers. h-split across two DMA
            # queues (sync / scalar) so arrival outruns consumption.
            for t in range(NT2):
                nc.sync.dma_start(
                    xst[0][t][:], xT[0:128, 512 * t:512 * (t + 1)])
                (nc.sync if t < 2 else nc.gpsimd).dma_start(
                    xst[1][t][:], xT[128:256, 512 * t:512 * (t + 1)])

            # kv partials are AllReduced in TWO halves: the first at P1's
            # midpoint, so only the second (smaller) transfer plus its
            # ~11us trigger latency must hide under stage C.
            kvb_in = [dram.tile([2, 128, 130], bf16, tag=f"kvb_in{h}",
                                name=f"kvb_in{h}") for h in range(2)]
            kvb_out = [dram.tile([2, 128, 130], bf16, tag=f"kvb_out{h}",
                                 name=f"kvb_out{h}") for h in range(2)]
            kvev = [[stash.tile([128, 130], bf16, tag=f"kvev{h}_{m}",
                                name=f"kvev{h}_{m}") for m in range(2)]
                    for h in range(2)]

            # ---- phase 1: k,v projection + phi(k) + KV/ksum accumulation ---
            # ps_qkv opens FIRST so it takes the low PSUM banks: phase-2's
            # ps_qt then aliases quickly-freed qkv space instead of the kvp
            # accumulators (whose eviction sits at the end of a busy engine
            # queue -- that alias cost a 7.6us tensor-engine stall at the
            # phase boundary).
            with tc.tile_pool(name="ps_qkv", bufs=3, space=PSUM) as ps_qkv, \
                 tc.tile_pool(name="ps_kv", bufs=1, space=PSUM) as ps_kv, \
                 tc.tile_pool(name="sb1", bufs=4) as sb1:
                kvp = [ps_kv.tile([128, 130], f32, tag=f"kv{m}",
                                  name=f"kvp{m}") for m in range(2)]

                def evict_half(hf):
                    for m in range(2):
                        nc.scalar.activation(kvev[hf][m][:], kvp[m][:],
                                             AF.Copy)
                        nc.sync.dma_start(kvb_in[hf][m], kvev[hf][m][:])
                    if collective:
                        nc.gpsimd.collective_compute(
                            "AllReduce", mybir.AluOpType.add,
                            replica_groups=[[2 * p, 2 * p + 1]
                                            for p in range(N_CORES // 2)],
                            ins=[kvb_in[hf][:].opt()],
                            outs=[kvb_out[hf][:].opt()])
                    else:
                        nc.sync.dma_start(kvb_out[hf][:], kvb_in[hf][:])

                def kv_accum(i):
                    s = i % 4
                    phik = phiks.pop(i)
                    for j in range(2):
                        g = i * 2 + j
                        for m in range(2):
                            nc.tensor.matmul(
                                kvp[m][:, :],
                                phik[:, j, 128 * m:128 * (m + 1)],
                                vbuf[:, s, j, m, :],
                                start=(g == 0 or g == NT1),
                                stop=(g == NT1 - 1 or g == 2 * NT1 - 1),
                                skip_group_check=True)

                phiks = {}
                # software pipeline: kv matmuls run two iterations behind
                # the qkv matmuls, so the tensor engine never waits on the
                # phi(k) scalar/vector chain (stalls there reset the PE
                # p-state ramp and halve the matmul clock).
                LAG = 2
                for i in range(NT1):
                    qkv_ps = ps_qkv.tile([128, 2, 512], f32, tag="qkv")
                    for j in range(2):
                        g = i * 2 + j             # global l-tile index
                        t, o = g // 4, (g % 4) * 128
                        for h in range(2):
                            nc.tensor.matmul(
                                qkv_ps[:, j, :],
                                xst[h][t][:, o:o + 128],
                                w_sb[h][:, C:3 * C],
                                start=(h == 0), stop=(h == 1))
                    if i >= LAG:
                        kv_accum(i - LAG)
                        if i - LAG == NT1 // 2 - 1:
                            evict_half(0)
                    kview = qkv_ps[:, :, 0:256]
                    vview = qkv_ps[:, :, 256:512].rearrange(
                        "p j (m c) -> p j m c", m=2)
                    e_k = sb1.tile([128, 2, 256], bf16, tag="e_k")
                    t_k = sb1.tile([128, 2, 256], bf16, tag="t_k")
                    phik = sb1.tile([128, 2, 256], bf16, tag="phik")
                    phiks[i] = phik
                    # phi(k) = elu(k)+1 = relu(k) + min(exp(k), 1):
                    # exp on Act; min-with-1 on DVE at 4x (all-bf16 SBUF);
                    # relu+add fused in one scalar_tensor_tensor pass.
                    nc.scalar.activation(e_k[:], kview, AF.Exp)
                    nc.vector.tensor_scalar(t_k[:], e_k[:], 1.0, 0.0,
                                            OP.min, OP.bypass)
                    nc.vector.scalar_tensor_tensor(phik[:], kview, 0.0,
                                                   t_k[:], OP.max, OP.add)
                    s = i % 4
                    # v psum->sbuf: ~2/3 on Scalar; every 3rd iteration
                    # splits across both engines (a full-DVE iteration
                    # exceeds the tensor time and stalls the PE)
                    if i % 3 != 2:
                        nc.scalar.activation(vbuf[:, s, :, :, 0:128],
                                             vview, AF.Copy)
                    else:
                        nc.scalar.activation(vbuf[:, s, :, 0, 0:128],
                                             vview[:, :, 0, :], AF.Copy)
                        nc.vector.tensor_copy(vbuf[:, s, :, 1, 0:128],
                                              vview[:, :, 1, :])
                for i in range(NT1 - LAG, NT1):
                    kv_accum(i)
                evict_half(1)

            # ---- phase 2 -------------------------------------------------
            # Stage C (all chunks): q^T projection + phi(q); hides the
            # AllReduce. The M/ksT build is emitted after chunk BUILDC (the
            # AR has landed by then) so stage D starts immediately after C.
            phiq_all = stash.tile([128, NT2, 2, 512], bf16, tag="phiq_all",
                                  name="phiq_all")
            # AllReduce result + derived operands
            kvr = [stash.tile([128, 130], bf16, tag=f"kvr{m}",
                              name=f"kvr{m}") for m in range(2)]
            kvrh = [[stash.tile([128, 130], bf16, tag=f"kvrh{h}_{m}",
                                name=f"kvrh{h}_{m}") for m in range(2)]
                    for h in range(2)]
            kvblkT = [stash.tile([128, 128], bf16, tag=f"kvblkT{m}",
                                 name=f"kvblkT{m}") for m in range(2)]
            M_sb = [stash.tile([128, C], bf16, tag=f"M{m}", name=f"M{m}")
                    for m in range(2)]
            ksx = [stash.tile([128, 128], bf16, tag=f"ksx{m}",
                              name=f"ksx{m}") for m in range(2)]

            def emit_build(ps_bld):
                """kv halves summed, -> kvblkT (PE transpose + mask),
                M = kvblkT.T @ Wp, ksx = ksum block-expanded. ~2us,
                overlapped with the tail of stage C."""
                for hf in range(2):
                    for m in range(2):
                        nc.sync.dma_start(kvrh[hf][m][:], kvb_out[hf][m])
                for m in range(2):
                    nc.vector.tensor_tensor(kvr[m][:], kvrh[0][m][:],
                                            kvrh[1][m][:], op=OP.add)
                for m in range(2):
                    tr = ps_bld.tile([128, 128], bf16, tag="tr")
                    nc.tensor.transpose(tr[:], kvr[m][:, 0:128], eyeid_sb[:])
                    # keep only the per-head diagonal blocks (off-diagonal
                    # entries are junk from the m-half outer product)
                    nc.vector.tensor_tensor(kvblkT[m][:], tr[:], eye_sb[:],
                                            op=OP.mult)
                    mps = ps_bld.tile([128, C], f32, tag="mps")
                    nc.tensor.matmul(mps[:], kvblkT[m][:], wp_sb[m][:],
                                     start=True, stop=True)
                    nc.scalar.activation(M_sb[m][:], mps[:], AF.Copy)
                    # ksx[p, o] = ksum[p] if p//32 == o//32 else 0: the den
                    # matmul then emits z already replicated to all 32
                    # partitions of its head (no separate broadcast needed)
                    ks32 = stash.tile([128, 1], f32, tag=f"ks32_{m}",
                                      name=f"ks32_{m}")
                    nc.vector.tensor_copy(ks32[:], kvr[m][:, 128:129])
                    nc.vector.tensor_scalar(ksx[m][:], eye_sb[:],
                                            ks32[:], 0.0,
                                            OP.mult, OP.add)

            with tc.tile_pool(name="ps_qt", bufs=3, space=PSUM) as ps_qt, \
                 tc.tile_pool(name="ps_bld", bufs=1, space=PSUM) as ps_bld, \
                 tc.tile_pool(name="sb2", bufs=3) as sb2:
                for c in range(NT2):
                    qt_ps = ps_qt.tile([128, 2, 512], f32, tag="qt")
                    for m in range(2):
                        for h in range(2):
                            nc.tensor.matmul(
                                qt_ps[:, m, :],
                                w_sb[h][:, 128 * m:128 * (m + 1)],
                                xst[h][c][:],
                                start=(h == 0), stop=(h == 1))
                    e_q = sb2.tile([128, 2, 512], bf16, tag="e_q")
                    t_q = sb2.tile([128, 2, 512], bf16, tag="t_q")
                    nc.scalar.activation(e_q[:], qt_ps[:], AF.Exp)
                    # phi(q) = relu(q) + min(exp(q), 1), as in phase 1
                    nc.vector.tensor_scalar(t_q[:], e_q[:], 1.0, 0.0,
                                            OP.min, OP.bypass)
                    nc.vector.scalar_tensor_tensor(phiq_all[:, c], qt_ps[:],
                                                   0.0, t_q[:],
                                                   OP.max, OP.add)
                    if c == BUILDC:
                        emit_build(ps_bld)

            # ---- stage D: den(broadcast) -> 1/z -> ys -> out = ysT.T @ M
            # Under the device-level DVFS throttle the PE clock is capped,
            # so total tensor CYCLES is the currency: den via ksx emits z
            # already replicated per head-block (1024cyc) and Mproj (2048)
            # is the only other PE work -- 25% fewer cycles than computing
            # y then proj. recip outputs bf16 so ys runs at DVE 2x.
            with tc.tile_pool(name="ps_dn", bufs=2, space=PSUM) as ps_dn, \
                 tc.tile_pool(name="ps_out", bufs=2, space=PSUM) as ps_out, \
                 tc.tile_pool(name="sb3", bufs=3) as sb3:

                dns, zexs, yss, outs = {}, {}, {}, {}

                def recip_s(out_ap, in_ap):
                    """Reciprocal on the scalar engine (table-based,
                    ~1.2e-5 rel err measured on HW; tolerance here is
                    2e-2, see module docstring)."""
                    eng = nc.scalar
                    ins = [eng.lower_ap(in_ap),
                           mybir.ImmediateValue(dtype=f32, value=0.0),
                           mybir.ImmediateValue(dtype=f32, value=1.0),
                           mybir.ImmediateValue(dtype=f32, value=0.0)]
                    eng.add_instruction(mybir.InstActivation(
                        name=nc.get_next_instruction_name(),
                        func=AF.Reciprocal,
                        ins=ins, outs=[eng.lower_ap(out_ap)]))

                def f_den(c):
                    dn = ps_dn.tile([128, 2, 512], f32, tag="dn")
                    dns[c] = dn
                    for m in range(2):
                        nc.tensor.matmul(dn[:, m, :],
                                         ksx[m][:], phiq_all[:, c, m, :],
                                         start=True, stop=True)

                def f_recip(c):
                    # lags den by one iteration: Act starts at iter-begin
                    # with no wait on this iter's den matmuls (the den-wait
                    # + recip + outcopy serial chain was the D wall)
                    zex = sb3.tile([128, 2, 512], bf16, tag="zex")
                    zexs[c] = zex
                    recip_s(zex[:], dns[c][:])

                def f_ys(c):
                    ys = sb3.tile([128, 2, 512], bf16, tag="ys")
                    yss[c] = ys
                    nc.vector.tensor_tensor(ys[:], phiq_all[:, c],
                                            zexs.pop(c)[:], op=OP.mult)
                    dns.pop(c)

                def f_proj(c):
                    out_ps = ps_out.tile([128, 4, 256], f32, tag="op")
                    outs[c] = out_ps
                    ys = yss.pop(c)
                    for j in range(4):
                        for m in range(2):
                            nc.tensor.matmul(
                                out_ps[:, j, :],
                                ys[:, m, 128 * j:128 * (j + 1)],
                                M_sb[m][:],
                                start=(m == 0),
                                stop=(m == 1 and not with_bias))
                        if with_bias:
                            nc.tensor.matmul(out_ps[:, j, :],
                                             ones_k1[:], brow[:],
                                             start=False, stop=True)

                def f_out(c):
                    out_ps = outs.pop(c)
                    out_sb = sb3.tile([128, 4, 256], bf16, tag="out_sb")
                    # psum->sbuf copy split Act/DVE so both stay under the
                    # tensor-engine time (keeps the PE the bottleneck)
                    pf = out_ps[:].rearrange("p a b -> p (a b)")
                    sf = out_sb[:].rearrange("p a b -> p (a b)")
                    nc.scalar.activation(sf[:, 0:320], pf[:, 0:320], AF.Copy)
                    nc.vector.tensor_copy(sf[:, 320:1024], pf[:, 320:1024])
                    # two DMA queues so output traffic doesn't bottleneck
                    nc.sync.dma_start(out_r[c][:, 0:2, :],
                                      out_sb[:, 0:2, :])
                    nc.gpsimd.dma_start(out_r[c][:, 2:4, :],
                                        out_sb[:, 2:4, :])

                for cc in range(NT2 + 2):
                    if cc < NT2:
                        f_den(cc)
                    if 1 <= cc <= NT2:
                        f_recip(cc - 1)
                    if cc >= 2:
                        f_proj(cc - 2)
                    if 1 <= cc <= NT2:
                        f_ys(cc - 1)
                    if cc >= 2:
                        f_out(cc - 2)

    if split_waits:
        _split_multiwaits(nc)
    return nc


def _split_multiwaits(nc, limit=1):
    """This container's walrus rejects instructions carrying more than a
    couple of sync waits (CoreV3 setupSyncWait: 'Too many sync wait
    commands'). Splitting extra waits onto preceding same-engine NoOps is
    semantically identical on an in-order engine."""
    from concourse import mybir

    f = nc.m.functions[0]
    for b in f.blocks:
        new_insts = []
        for inst in b.instructions:
            si = getattr(inst, "sync_info", None)
            waits = list(si.on_wait) if (si and si.on_wait) else []
            if len(waits) > limit:
                head, keep = waits[:-limit], waits[-limit:]
                for w0 in range(0, len(head), limit):
                    nop = mybir.InstNoOp(
                        name=nc.get_next_instruction_name(), ins=[], outs=[])
                    nop.engine = inst.engine
                    nop.sync_info = mybir.SyncInfo(
                        on_wait=head[w0:w0 + limit], on_update=[])
                    new_insts.append(nop)
                inst.sync_info = mybir.SyncInfo(
                    on_wait=keep, on_update=list(si.on_update or []))
            new_insts.append(inst)
        b.instructions[:] = new_insts


def _build_null_nc(lh=LH):
    """Minimal program with the same I/O signature (for dispatch-overhead
    measurement in test.py)."""
    import concourse.bass as bass
    import concourse.mybir as mybir
    import concourse.tile as tile

    f32 = mybir.dt.float32
    bf16 = mybir.dt.bfloat16
    nc = bass.Bass("TRN2", target_bir_lowering=False, debug=False,
                   num_devices=N_CORES)
    xT = nc.dram_tensor("xT", [C, lh], bf16, kind="ExternalInput")
    nc.dram_tensor("w_qkv", [C, 3 * C], bf16, kind="ExternalInput")
    nc.dram_tensor("w_proj", [C, C], bf16, kind="ExternalInput")
    nc.dram_tensor("b_proj", [1, C], bf16, kind="ExternalInput")
    nc.dram_tensor("eye32", [128, 128], bf16, kind="ExternalInput")
    nc.dram_tensor("eye128", [128, 128], bf16, kind="ExternalInput")
    out = nc.dram_tensor("out", [lh, C], bf16, kind="ExternalOutput")
    with tile.TileContext(nc) as tc:
        with tc.tile_pool(name="p", bufs=1) as p:
            t = p.tile([1, 512], bf16, tag="t", name="t")
            nc.sync.dma_start(t[:], xT[0:1, 0:512])
            nc.sync.dma_start(out[0:1, :], t[:])
    _split_multiwaits(nc)
    return nc


class _Runner:
    """Cached jit(shard_map(bass_exec)) over the 8 axon trn2 cores."""

    def __init__(self, nc):
        import jax
        import jax.numpy as jnp
        from jax.sharding import Mesh, PartitionSpec
        from jax.experimental.shard_map import shard_map
        import concourse.mybir as mybir
        from concourse import bass2jax

        bass2jax.install_neuronx_cc_hook()
        self.jax, self.jnp = jax, jnp

        partition_name = (nc.partition_id_tensor.name
                          if nc.partition_id_tensor else None)
        in_names, out_names, out_avals = [], [], []
        for alloc in nc.m.functions[0].allocations:
            if not isinstance(alloc, mybir.MemoryLocationSet):
                continue
            name = alloc.memorylocations[0].name
            if alloc.kind == "ExternalInput":
                if name != partition_name:
                    in_names.append(name)
            elif alloc.kind == "ExternalOutput":
                out_names.append(name)
                out_avals.append(jax.core.ShapedArray(
                    tuple(alloc.tensor_shape), mybir.dt.np(alloc.dtype)))
        assert nc.dbg_addr is None
        self.in_names, self.out_names, self.out_avals = in_names, out_names, out_avals
        n_params = len(in_names)
        all_in_names = in_names + out_names
        if partition_name is not None:
            all_in_names = all_in_names + [partition_name]
        all_in_names = tuple(all_in_names)

        def _body(*args):
            operands = list(args)
            if partition_name is not None:
                operands.append(bass2jax.partition_id_tensor())
            outs = bass2jax._bass_exec_p.bind(
                *operands,
                out_avals=tuple(out_avals),
                in_names=all_in_names,
                out_names=tuple(out_names),
                lowering_input_output_aliases=(),
                sim_require_finite=True,
                sim_require_nnan=True,
                nc=nc,
            )
            return tuple(outs)

        devices = jax.devices()[:N_CORES]
        self.mesh = Mesh(np.asarray(devices), ("core",))
        spec = PartitionSpec("core")
        n_outs = len(out_names)
        self.donate = tuple(range(n_params, n_params + n_outs))
        self.fn = jax.jit(
            shard_map(_body, mesh=self.mesh, in_specs=(spec,) * (n_params + n_outs),
                      out_specs=(spec,) * n_outs, check_rep=False),
            donate_argnums=self.donate, keep_unused=True)
        self.sharding = jax.sharding.NamedSharding(self.mesh, spec)

        def _zeros():
            return tuple(
                jnp.zeros((N_CORES * a.shape[0], *a.shape[1:]), a.dtype)
                for a in out_avals)
        self.zeros_fn = jax.jit(_zeros, out_shardings=(self.sharding,) * n_outs)

    def place_inputs(self, in_maps):
        concat = [np.concatenate([np.asarray(m[n]) for m in in_maps], axis=0)
                  for n in self.in_names]
        return [self.jax.device_put(a, self.sharding) for a in concat]

    def call(self, dev_in):
        outs = self.fn(*dev_in, *self.zeros_fn())
        self.jax.block_until_ready(outs)
        return outs

    def run(self, in_maps):
        outs = self.call(self.place_inputs(in_maps))
        res = []
        for c in range(N_CORES):
            res.append({n: np.asarray(outs[i]).reshape(
                N_CORES, *self.out_avals[i].shape)[c]
                for i, n in enumerate(self.out_names)})
        return res


def _get_runner(lh=LH, with_bias=False, null=False, repeat=1):
    key = (lh, with_bias, null, repeat)
    if key not in _NC_CACHE:
        nc = (_build_null_nc(lh) if null
              else _build_nc(lh, with_bias, repeat=repeat))
        _NC_CACHE[key] = _Runner(nc)
    return _NC_CACHE[key]


def _make_eye32():
    return np.kron(np.eye(4, dtype=np.float32), np.ones((32, 32), np.float32))


def _make_in_maps(x, W_qkv, W_proj, b_proj, lh=LH):
    import ml_dtypes
    bf = ml_dtypes.bfloat16
    ncores_b = B * (L // lh)
    xb = x.reshape(B, L // lh, lh, C)
    eye = _make_eye32().astype(bf)
    eyeid = np.eye(128, dtype=np.float32).astype(bf)
    w = np.ascontiguousarray(W_qkv).astype(bf)
    wp = np.ascontiguousarray(W_proj).astype(bf)
    bp = np.ascontiguousarray(b_proj).reshape(1, C).astype(bf)
    in_maps = []
    for c in range(ncores_b):
        bb, hh = divmod(c, L // lh)
        xTc = np.ascontiguousarray(xb[bb, hh].T).astype(bf)  # [C, lh]
        in_maps.append({"xT": xTc, "w_qkv": w, "w_proj": wp, "b_proj": bp,
                        "eye32": eye, "eye128": eyeid})
    return in_maps


def _assemble(results):
    outs = [results[c]["out"] for c in range(N_CORES)]
    y = np.stack(outs).reshape(B, 2, LH, C).reshape(B, L, C)
    return np.ascontiguousarray(y.reshape(BV, HW, C), dtype=np.float32)


def _run(x, W_qkv, W_proj, b_proj):
    with_bias = bool(np.any(b_proj))
    runner = _get_runner(LH, with_bias)
    in_maps = _make_in_maps(x, W_qkv, W_proj, b_proj)
    return _assemble(runner.run(in_maps))


def kernel(x, W_qkv, W_proj, b_proj):
    return _run(np.asarray(x, np.float32), np.asarray(W_qkv, np.float32),
                np.asarray(W_proj, np.float32), np.asarray(b_proj, np.float32))
